# revision 2
# baseline (speedup 1.0000x reference)
"""Trainium2 Bass kernel for BaselineMoE (top-6-of-32 routed experts + 2 shared).

Strategy (8 NeuronCores, expert-parallel per the sharding hint):
  - Host computes the (cheap) router softmax/top-k from the actual inputs,
    gathers each expert's tokens into a padded, transposed buffer, and deals
    the 32 routed experts across 8 cores x 4 slots, balancing per-core load.
  - Each core runs a dense SwiGLU MLP (gate/up/down, sigmoid(gate)*up) for its
    4 routed experts on the pre-gathered tokens, with the per-token top-k gate
    weights applied on-device during PSUM evacuation. The 2 shared experts are
    replicated and run data-parallel on a 256-token shard per core.
  - Weights/activations are cast to bf16 on host (halves HBM traffic; PSUM
    accumulation stays f32). Expert outputs come back f32; the host
    scatter-adds them into the residual stream.

Capacities (per-slot token counts) are computed from the actual routing at
call time, so the emitted program adapts to the input. All matmuls are
[128x128] stationary x [128, C<=512] moving, PSUM-accumulated over the
contraction dim.
"""

import numpy as np
import ml_dtypes

import concourse.bacc as bacc
import concourse.tile as tile
import concourse.mybir as mybir
from concourse.bass_utils import run_bass_kernel_spmd

H = 2048
I = 1024
E = 32
NS = 2
TOP_K = 6
SCALE = 1.0
NCORES = 8
SLOTS = 4          # routed experts per core
TSH = 256          # shared-expert tokens per core (T / NCORES)
KH = H // 128      # 16 k-tiles over H
KI = I // 128      # 8 k-tiles over I
BF16 = mybir.dt.bfloat16
F32 = mybir.dt.float32

_PROGRAM_CACHE: dict = {}


def _to_bf16(a: np.ndarray) -> np.ndarray:
    """f32 -> bf16 with round-to-nearest-even (fast uint trick)."""
    a = np.ascontiguousarray(a, dtype=np.float32)
    u = a.view(np.uint32)
    r = (u + np.uint32(0x7FFF) + ((u >> np.uint32(16)) & np.uint32(1))) >> np.uint32(16)
    return r.astype(np.uint16).view(ml_dtypes.bfloat16)


def _route(flat: np.ndarray, Wr: np.ndarray):
    """Host router: softmax over experts, exact top-k gate mask."""
    logits = flat.astype(np.float32) @ Wr.astype(np.float32)
    m = logits.max(axis=-1, keepdims=True)
    p = np.exp(logits - m)
    p /= p.sum(axis=-1, keepdims=True)
    T = p.shape[0]
    idx = np.argpartition(-p, TOP_K - 1, axis=-1)[:, :TOP_K]
    gates = np.zeros((T, E), np.float32)
    rows = np.arange(T)[:, None]
    gates[rows, idx] = p[rows, idx] * SCALE
    return gates


def _assign_experts(counts: np.ndarray):
    """Deal experts into (core, slot) balancing per-core token totals.

    Slot s holds the experts ranked [8s, 8s+8) by token count; within a slot
    the largest expert goes to the least-loaded core.
    """
    order = np.argsort(-counts, kind="stable")
    assign = [[-1] * SLOTS for _ in range(NCORES)]
    load = np.zeros(NCORES, np.int64)
    caps = []
    for s in range(SLOTS):
        group = list(order[s * NCORES:(s + 1) * NCORES])
        caps.append(int(counts[group].max()) if group else 0)
        for e in group:  # descending count; give to least-loaded core
            c = int(np.argmin(load))
            assign[c][s] = int(e)
            load[c] += counts[e]
    caps = [max(64, -(-c // 32) * 32) for c in caps]  # round up to 32, min 64
    return assign, caps


def _mlp_phases(nc, pools, w_dram_gate, w_dram_up, w_dram_down, xg_t, C, gb, y_dram,
                zs_out=None, skip_down=False):
    """Emit one SwiGLU expert: gate->sigmoid, up->mul, down->evacuate.

    If skip_down, stops after producing z (bf16 [128, KI, C]) and returns it.
    """
    wpool, sgpool, zpool, opool, pgpool, pupool, pypool = pools

    wg_t = wpool.tile([128, KH, I], BF16, tag="w")
    for k in range(KH):
        nc.sync.dma_start(wg_t[:, k, :], w_dram_gate[k])
    sg = sgpool.tile([128, KI, C], F32, tag="sg")
    for mm in range(KI):
        pg = pgpool.tile([128, C], F32, tag="pg")
        for k in range(KH):
            nc.tensor.matmul(pg[:], wg_t[:, k, mm * 128:(mm + 1) * 128],
                             xg_t[:, k, :], start=(k == 0), stop=(k == KH - 1))
        nc.scalar.activation(sg[:, mm, :], pg[:],
                             mybir.ActivationFunctionType.Sigmoid)

    wu_t = wpool.tile([128, KH, I], BF16, tag="w")
    for k in range(KH):
        nc.sync.dma_start(wu_t[:, k, :], w_dram_up[k])
    z = zpool.tile([128, KI, C], BF16, tag="z")
    for mm in range(KI):
        pu = pupool.tile([128, C], F32, tag="pu")
        for k in range(KH):
            nc.tensor.matmul(pu[:], wu_t[:, k, mm * 128:(mm + 1) * 128],
                             xg_t[:, k, :], start=(k == 0), stop=(k == KH - 1))
        nc.vector.tensor_mul(z[:, mm, :], sg[:, mm, :], pu[:])

    if skip_down:
        return z

    wd_t = wpool.tile([128, KI, H], BF16, tag="w")
    for j in range(KI):
        nc.sync.dma_start(wd_t[:, j, :], w_dram_down[j])
    for h in range(KH):
        py = pypool.tile([128, C], F32, tag="py")
        for j in range(KI):
            nc.tensor.matmul(py[:], wd_t[:, j, h * 128:(h + 1) * 128],
                             z[:, j, :], start=(j == 0), stop=(j == KI - 1))
        ot = opool.tile([128, C], F32, tag="o")
        if gb is not None:
            nc.vector.tensor_mul(ot[:], py[:], gb[:])
        else:
            nc.vector.tensor_copy(ot[:], py[:])
        nc.sync.dma_start(y_dram[h], ot[:])
    return None


def build_program(caps):
    """Build the per-core Bass program for the given slot capacities."""
    caps = tuple(int(c) for c in caps)
    if caps in _PROGRAM_CACHE:
        return _PROGRAM_CACHE[caps]

    nc = bacc.Bacc("TRN2", target_bir_lowering=False, debug=False)

    xg_d, wg_d, wu_d, wd_d, g_d, y_d = [], [], [], [], [], []
    for s in range(SLOTS):
        C = caps[s]
        xg_d.append(nc.dram_tensor(f"xg{s}", [KH, 128, C], BF16, kind="ExternalInput"))
        wg_d.append(nc.dram_tensor(f"wg{s}", [KH, 128, I], BF16, kind="ExternalInput"))
        wu_d.append(nc.dram_tensor(f"wu{s}", [KH, 128, I], BF16, kind="ExternalInput"))
        wd_d.append(nc.dram_tensor(f"wd{s}", [KI, 128, H], BF16, kind="ExternalInput"))
        g_d.append(nc.dram_tensor(f"g{s}", [1, C], F32, kind="ExternalInput"))
        y_d.append(nc.dram_tensor(f"y{s}", [KH, 128, C], F32, kind="ExternalOutput"))
    xs_d = nc.dram_tensor("xs", [KH, 128, TSH], BF16, kind="ExternalInput")
    wgs_d = nc.dram_tensor("wgs", [NS, KH, 128, I], BF16, kind="ExternalInput")
    wus_d = nc.dram_tensor("wus", [NS, KH, 128, I], BF16, kind="ExternalInput")
    wds_d = nc.dram_tensor("wds", [NS, KI, 128, H], BF16, kind="ExternalInput")
    ys_d = nc.dram_tensor("ys", [KH, 128, TSH], F32, kind="ExternalOutput")

    with tile.TileContext(nc) as tc:
        with (
            tc.tile_pool(name="w", bufs=3) as wpool,
            tc.tile_pool(name="xg", bufs=2) as xpool,
            tc.tile_pool(name="gb", bufs=2) as gbpool,
            tc.tile_pool(name="sg", bufs=2) as sgpool,
            tc.tile_pool(name="z", bufs=2) as zpool,
            tc.tile_pool(name="o", bufs=4) as opool,
            tc.tile_pool(name="pg", bufs=2, space="PSUM") as pgpool,
            tc.tile_pool(name="pu", bufs=2, space="PSUM") as pupool,
            tc.tile_pool(name="py", bufs=4, space="PSUM") as pypool,
        ):
            pools = (wpool, sgpool, zpool, opool, pgpool, pupool, pypool)

            for s in range(SLOTS):
                C = caps[s]
                xg_t = xpool.tile([128, KH, C], BF16, tag="xg")
                for k in range(KH):
                    nc.sync.dma_start(xg_t[:, k, :], xg_d[s][k])
                gb = gbpool.tile([128, C], F32, tag="gb")
                nc.sync.dma_start(gb[:], g_d[s][:].partition_broadcast(128))
                _mlp_phases(nc, pools, wg_d[s], wu_d[s], wd_d[s],
                            xg_t, C, gb, y_d[s])

            # shared experts: both MLPs, down-proj accumulated into one PSUM
            xs_t = xpool.tile([128, KH, TSH], BF16, tag="xg")
            for k in range(KH):
                nc.sync.dma_start(xs_t[:, k, :], xs_d[k])
            zs = []
            for e in range(NS):
                zs.append(_mlp_phases(nc, pools, wgs_d[e], wus_d[e], None,
                                      xs_t, TSH, None, None, skip_down=True))
            wd_ts = []
            for e in range(NS):
                wd_t = wpool.tile([128, KI, H], BF16, tag="w")
                for j in range(KI):
                    nc.sync.dma_start(wd_t[:, j, :], wds_d[e][j])
                wd_ts.append(wd_t)
            for h in range(KH):
                py = pypool.tile([128, TSH], F32, tag="py")
                n_acc = NS * KI
                acc = 0
                for e in range(NS):
                    for j in range(KI):
                        nc.tensor.matmul(py[:], wd_ts[e][:, j, h * 128:(h + 1) * 128],
                                         zs[e][:, j, :], start=(acc == 0),
                                         stop=(acc == n_acc - 1))
                        acc += 1
                ot = opool.tile([128, TSH], F32, tag="o")
                nc.vector.tensor_copy(ot[:], py[:])
                nc.sync.dma_start(ys_d[h], ot[:])

    nc.compile()
    _PROGRAM_CACHE[caps] = nc
    return nc


def prepare(x, Wr, Wg_s, Wu_s, Wd_s, Wg, Wu, Wd):
    """Host-side routing, sharding and bf16 packing. Returns (nc, in_maps, meta)."""
    flat = np.ascontiguousarray(x, np.float32).reshape(-1, H)
    T = flat.shape[0]
    assert T == NCORES * TSH

    gates = _route(flat, Wr)
    tok_idx = [np.nonzero(gates[:, e])[0].astype(np.int32) for e in range(E)]
    counts = np.array([len(ix) for ix in tok_idx], np.int64)
    assign, caps = _assign_experts(counts)

    nc = build_program(caps)

    xT = np.ascontiguousarray(flat.T)          # [H, T] f32
    Wg_b = _to_bf16(Wg)                        # [E, H, I]
    Wu_b = _to_bf16(Wu)
    Wd_b = _to_bf16(Wd)
    wgs = _to_bf16(Wg_s).reshape(NS, KH, 128, I)
    wus = _to_bf16(Wu_s).reshape(NS, KH, 128, I)
    wds = _to_bf16(Wd_s).reshape(NS, KI, 128, H)

    in_maps = []
    for c in range(NCORES):
        im = {"wgs": wgs, "wus": wus, "wds": wds}
        im["xs"] = _to_bf16(xT[:, c * TSH:(c + 1) * TSH]).reshape(KH, 128, TSH)
        for s in range(SLOTS):
            e = assign[c][s]
            ix = tok_idx[e]
            C = caps[s]
            xg = np.zeros((H, C), ml_dtypes.bfloat16)
            xg[:, :len(ix)] = _to_bf16(xT[:, ix])
            im[f"xg{s}"] = xg.reshape(KH, 128, C)
            g = np.zeros((1, C), np.float32)
            g[0, :len(ix)] = gates[ix, e]
            im[f"g{s}"] = g
            im[f"wg{s}"] = Wg_b[e].reshape(KH, 128, I)
            im[f"wu{s}"] = Wu_b[e].reshape(KH, 128, I)
            im[f"wd{s}"] = Wd_b[e].reshape(KI, 128, H)
        in_maps.append(im)

    meta = {"assign": assign, "caps": caps, "tok_idx": tok_idx,
            "flat": flat, "shape": x.shape}
    return nc, in_maps, meta


def postprocess(results, meta):
    """Scatter-add per-expert outputs + shared shards + residual."""
    flat = meta["flat"]
    T = flat.shape[0]
    out = flat.copy()
    for c in range(NCORES):
        sh = results[c]["ys"].reshape(H, TSH)
        out[c * TSH:(c + 1) * TSH] += sh.T
        for s in range(SLOTS):
            e = meta["assign"][c][s]
            ix = meta["tok_idx"][e]
            Y = results[c][f"y{s}"].reshape(H, meta["caps"][s])
            out[ix] += Y[:, :len(ix)].T
    return out.reshape(meta["shape"]).astype(np.float32, copy=False)


def kernel(x, Wr, Wg_s, Wu_s, Wd_s, Wg, Wu, Wd):
    nc, in_maps, meta = prepare(x, Wr, Wg_s, Wu_s, Wd_s, Wg, Wu, Wd)
    res = run_bass_kernel_spmd(nc, in_maps, list(range(NCORES)))
    return postprocess(res.results, meta)


# revision 6
# speedup vs baseline: 852.1448x; 852.1448x over previous
"""Trainium2 Bass kernel for BaselineMoE (top-6-of-32 routed experts + 2 shared).

Strategy (8 NeuronCores, expert-parallel per the sharding hint):
  - Host computes the (cheap) router softmax/top-k from the actual inputs,
    gathers each expert's tokens into a padded, transposed buffer, and deals
    the 32 routed experts across 8 cores x 4 slots, balancing per-core load.
  - Each core runs a dense SwiGLU MLP (gate/up/down, sigmoid(gate)*up) for its
    4 routed experts on the pre-gathered tokens, with the per-token top-k gate
    weights applied on-device during PSUM evacuation. The 2 shared experts are
    replicated and run data-parallel on a 256-token shard per core.
  - Weights/activations are cast to bf16 on host (halves HBM traffic; PSUM
    accumulation stays f32). Expert outputs come back f32; the host
    scatter-adds them into the residual stream.

Capacities (per-slot token counts) are computed from the actual routing at
call time, so the emitted program adapts to the input. All matmuls are
[128x128] stationary x [128, C<=512] moving, PSUM-accumulated over the
contraction dim.
"""

import numpy as np
import ml_dtypes

import concourse.bacc as bacc
import concourse.tile as tile
import concourse.mybir as mybir
from concourse.bass_utils import run_bass_kernel_spmd

H = 2048
I = 1024
E = 32
NS = 2
TOP_K = 6
SCALE = 1.0
NCORES = 8
SLOTS = 4          # routed experts per core
TSH = 256          # shared-expert tokens per core (T / NCORES)
KH = H // 128      # 16 k-tiles over H
KI = I // 128      # 8 k-tiles over I
BF16 = mybir.dt.bfloat16
F32 = mybir.dt.float32

_PROGRAM_CACHE: dict = {}


def _to_bf16(a: np.ndarray) -> np.ndarray:
    """f32 -> bf16 with round-to-nearest-even (fast uint trick)."""
    a = np.ascontiguousarray(a, dtype=np.float32)
    u = a.view(np.uint32)
    r = (u + np.uint32(0x7FFF) + ((u >> np.uint32(16)) & np.uint32(1))) >> np.uint32(16)
    return r.astype(np.uint16).view(ml_dtypes.bfloat16)


def _route(flat: np.ndarray, Wr: np.ndarray):
    """Host router: softmax over experts, exact top-k gate mask."""
    logits = flat.astype(np.float32) @ Wr.astype(np.float32)
    m = logits.max(axis=-1, keepdims=True)
    p = np.exp(logits - m)
    p /= p.sum(axis=-1, keepdims=True)
    T = p.shape[0]
    idx = np.argpartition(-p, TOP_K - 1, axis=-1)[:, :TOP_K]
    gates = np.zeros((T, E), np.float32)
    rows = np.arange(T)[:, None]
    gates[rows, idx] = p[rows, idx] * SCALE
    return gates


def _assign_experts(counts: np.ndarray):
    """Deal experts into (core, slot) balancing per-core token totals.

    Slot s holds the experts ranked [8s, 8s+8) by token count; within a slot
    the largest expert goes to the least-loaded core.
    """
    order = np.argsort(-counts, kind="stable")
    assign = [[-1] * SLOTS for _ in range(NCORES)]
    load = np.zeros(NCORES, np.int64)
    caps = []
    for s in range(SLOTS):
        group = list(order[s * NCORES:(s + 1) * NCORES])
        caps.append(int(counts[group].max()) if group else 0)
        for e in group:  # descending count; give to least-loaded core
            c = int(np.argmin(load))
            assign[c][s] = int(e)
            load[c] += counts[e]
    caps = [max(64, -(-c // 32) * 32) for c in caps]  # round up to 32, min 64
    return assign, caps


def _mlp_phases(nc, pools, w_dram_gate, w_dram_up, w_dram_down, xg_t, C, gb, y_dram,
                zs_out=None, skip_down=False):
    """Emit one SwiGLU expert: gate->sigmoid, up->mul, down->evacuate.

    If skip_down, stops after producing z (bf16 [128, KI, C]) and returns it.
    """
    wpool, sgpool, zpool, opool, pgpool, pupool, pypool = pools

    wg_t = wpool.tile([128, KH, I], BF16, tag="w")
    for k in range(KH):
        nc.sync.dma_start(wg_t[:, k, :], w_dram_gate[k])
    sg = sgpool.tile([128, KI, C], F32, tag="sg")
    for mm in range(KI):
        pg = pgpool.tile([128, C], F32, tag="pg")
        for k in range(KH):
            nc.tensor.matmul(pg[:], wg_t[:, k, mm * 128:(mm + 1) * 128],
                             xg_t[:, k, :], start=(k == 0), stop=(k == KH - 1))
        nc.scalar.activation(sg[:, mm, :], pg[:],
                             mybir.ActivationFunctionType.Sigmoid)

    wu_t = wpool.tile([128, KH, I], BF16, tag="w")
    for k in range(KH):
        nc.sync.dma_start(wu_t[:, k, :], w_dram_up[k])
    z = zpool.tile([128, KI, C], BF16, tag="z")
    for mm in range(KI):
        pu = pupool.tile([128, C], F32, tag="pu")
        for k in range(KH):
            nc.tensor.matmul(pu[:], wu_t[:, k, mm * 128:(mm + 1) * 128],
                             xg_t[:, k, :], start=(k == 0), stop=(k == KH - 1))
        nc.vector.tensor_mul(z[:, mm, :], sg[:, mm, :], pu[:])

    if skip_down:
        return z

    wd_t = wpool.tile([128, KI, H], BF16, tag="w")
    for j in range(KI):
        nc.sync.dma_start(wd_t[:, j, :], w_dram_down[j])
    for h in range(KH):
        py = pypool.tile([128, C], F32, tag="py")
        for j in range(KI):
            nc.tensor.matmul(py[:], wd_t[:, j, h * 128:(h + 1) * 128],
                             z[:, j, :], start=(j == 0), stop=(j == KI - 1))
        ot = opool.tile([128, C], F32, tag="o")
        if gb is not None:
            nc.vector.tensor_mul(ot[:], py[:], gb[:])
        else:
            nc.vector.tensor_copy(ot[:], py[:])
        nc.sync.dma_start(y_dram[h], ot[:])
    return None


def build_program(caps, loop_reps=None):
    """Build the per-core Bass program for the given slot capacities.

    loop_reps: if set, wrap the whole body in a device-side For_i loop —
    used by the test harness to amplify exec time above dispatch overhead.
    """
    caps = tuple(int(c) for c in caps)
    key = (caps, loop_reps)
    if key in _PROGRAM_CACHE:
        return _PROGRAM_CACHE[key]

    nc = bacc.Bacc("TRN2", target_bir_lowering=False, debug=False)

    xg_d, wg_d, wu_d, wd_d, g_d, y_d = [], [], [], [], [], []
    for s in range(SLOTS):
        C = caps[s]
        xg_d.append(nc.dram_tensor(f"xg{s}", [KH, 128, C], BF16, kind="ExternalInput"))
        wg_d.append(nc.dram_tensor(f"wg{s}", [KH, 128, I], BF16, kind="ExternalInput"))
        wu_d.append(nc.dram_tensor(f"wu{s}", [KH, 128, I], BF16, kind="ExternalInput"))
        wd_d.append(nc.dram_tensor(f"wd{s}", [KI, 128, H], BF16, kind="ExternalInput"))
        g_d.append(nc.dram_tensor(f"g{s}", [1, C], F32, kind="ExternalInput"))
        y_d.append(nc.dram_tensor(f"y{s}", [KH, 128, C], F32, kind="ExternalOutput"))
    xs_d = nc.dram_tensor("xs", [KH, 128, TSH], BF16, kind="ExternalInput")
    wgs_d = nc.dram_tensor("wgs", [NS, KH, 128, I], BF16, kind="ExternalInput")
    wus_d = nc.dram_tensor("wus", [NS, KH, 128, I], BF16, kind="ExternalInput")
    wds_d = nc.dram_tensor("wds", [NS, KI, 128, H], BF16, kind="ExternalInput")
    ys_d = nc.dram_tensor("ys", [KH, 128, TSH], F32, kind="ExternalOutput")

    from contextlib import ExitStack

    with tile.TileContext(nc) as tc:
        with (
            tc.tile_pool(name="w", bufs=3) as wpool,
            tc.tile_pool(name="xg", bufs=2) as xpool,
            tc.tile_pool(name="gb", bufs=2) as gbpool,
            tc.tile_pool(name="sg", bufs=2) as sgpool,
            tc.tile_pool(name="z", bufs=2) as zpool,
            tc.tile_pool(name="o", bufs=4) as opool,
            tc.tile_pool(name="pg", bufs=2, space="PSUM") as pgpool,
            tc.tile_pool(name="pu", bufs=2, space="PSUM") as pupool,
            tc.tile_pool(name="py", bufs=4, space="PSUM") as pypool,
            ExitStack() as stack,
        ):
            if loop_reps is not None:
                stack.enter_context(tc.For_i(0, loop_reps, 1))
            pools = (wpool, sgpool, zpool, opool, pgpool, pupool, pypool)

            for s in range(SLOTS):
                C = caps[s]
                xg_t = xpool.tile([128, KH, C], BF16, tag="xg")
                for k in range(KH):
                    nc.sync.dma_start(xg_t[:, k, :], xg_d[s][k])
                gb = gbpool.tile([128, C], F32, tag="gb")
                nc.sync.dma_start(gb[:], g_d[s][:].partition_broadcast(128))
                _mlp_phases(nc, pools, wg_d[s], wu_d[s], wd_d[s],
                            xg_t, C, gb, y_d[s])

            # shared experts: both MLPs, down-proj accumulated into one PSUM
            xs_t = xpool.tile([128, KH, TSH], BF16, tag="xg")
            for k in range(KH):
                nc.sync.dma_start(xs_t[:, k, :], xs_d[k])
            zs = []
            for e in range(NS):
                zs.append(_mlp_phases(nc, pools, wgs_d[e], wus_d[e], None,
                                      xs_t, TSH, None, None, skip_down=True))
            wd_ts = []
            for e in range(NS):
                wd_t = wpool.tile([128, KI, H], BF16, tag="w")
                for j in range(KI):
                    nc.sync.dma_start(wd_t[:, j, :], wds_d[e][j])
                wd_ts.append(wd_t)
            for h in range(KH):
                py = pypool.tile([128, TSH], F32, tag="py")
                n_acc = NS * KI
                acc = 0
                for e in range(NS):
                    for j in range(KI):
                        nc.tensor.matmul(py[:], wd_ts[e][:, j, h * 128:(h + 1) * 128],
                                         zs[e][:, j, :], start=(acc == 0),
                                         stop=(acc == n_acc - 1))
                        acc += 1
                ot = opool.tile([128, TSH], F32, tag="o")
                nc.vector.tensor_copy(ot[:], py[:])
                nc.sync.dma_start(ys_d[h], ot[:])

    nc.compile()
    _PROGRAM_CACHE[key] = nc
    return nc


def prepare(x, Wr, Wg_s, Wu_s, Wd_s, Wg, Wu, Wd):
    """Host-side routing, sharding and bf16 packing. Returns (nc, in_maps, meta)."""
    flat = np.ascontiguousarray(x, np.float32).reshape(-1, H)
    T = flat.shape[0]
    assert T == NCORES * TSH

    gates = _route(flat, Wr)
    tok_idx = [np.nonzero(gates[:, e])[0].astype(np.int32) for e in range(E)]
    counts = np.array([len(ix) for ix in tok_idx], np.int64)
    assign, caps = _assign_experts(counts)

    nc = build_program(caps)

    xT = np.ascontiguousarray(flat.T)          # [H, T] f32
    Wg_b = _to_bf16(Wg)                        # [E, H, I]
    Wu_b = _to_bf16(Wu)
    Wd_b = _to_bf16(Wd)
    wgs = _to_bf16(Wg_s).reshape(NS, KH, 128, I)
    wus = _to_bf16(Wu_s).reshape(NS, KH, 128, I)
    wds = _to_bf16(Wd_s).reshape(NS, KI, 128, H)

    in_maps = []
    for c in range(NCORES):
        im = {"wgs": wgs, "wus": wus, "wds": wds}
        im["xs"] = _to_bf16(xT[:, c * TSH:(c + 1) * TSH]).reshape(KH, 128, TSH)
        for s in range(SLOTS):
            e = assign[c][s]
            ix = tok_idx[e]
            C = caps[s]
            xg = np.zeros((H, C), ml_dtypes.bfloat16)
            xg[:, :len(ix)] = _to_bf16(xT[:, ix])
            im[f"xg{s}"] = xg.reshape(KH, 128, C)
            g = np.zeros((1, C), np.float32)
            g[0, :len(ix)] = gates[ix, e]
            im[f"g{s}"] = g
            im[f"wg{s}"] = Wg_b[e].reshape(KH, 128, I)
            im[f"wu{s}"] = Wu_b[e].reshape(KH, 128, I)
            im[f"wd{s}"] = Wd_b[e].reshape(KI, 128, H)
        in_maps.append(im)

    meta = {"assign": assign, "caps": caps, "tok_idx": tok_idx,
            "flat": flat, "shape": x.shape}
    return nc, in_maps, meta


def postprocess(results, meta):
    """Scatter-add per-expert outputs + shared shards + residual."""
    flat = meta["flat"]
    T = flat.shape[0]
    out = flat.copy()
    for c in range(NCORES):
        sh = results[c]["ys"].reshape(H, TSH)
        out[c * TSH:(c + 1) * TSH] += sh.T
        for s in range(SLOTS):
            e = meta["assign"][c][s]
            ix = meta["tok_idx"][e]
            Y = results[c][f"y{s}"].reshape(H, meta["caps"][s])
            out[ix] += Y[:, :len(ix)].T
    return out.reshape(meta["shape"]).astype(np.float32, copy=False)


def kernel(x, Wr, Wg_s, Wu_s, Wd_s, Wg, Wu, Wd):
    nc, in_maps, meta = prepare(x, Wr, Wg_s, Wu_s, Wd_s, Wg, Wu, Wd)
    res = run_bass_kernel_spmd(nc, in_maps, list(range(NCORES)))
    return postprocess(res.results, meta)
